# revision 1
# baseline (speedup 1.0000x reference)
"""Inverse Haar DWT2 (pywt 'haar' idwt2 convention) on 8 Trainium2 cores.

Input  x: [16, 256, 128, 128] f32 — 4 stacked subbands (LL|LH|HL|HH) of 64
channels each.  Output: [16, 64, 256, 256] f32.

Sharding: batch dim (16) split across 8 cores, 2 batches per core.  The
transform is elementwise per (batch, channel) — no communication.

Per-core kernel (x_loc [2, 256, 128, 128] -> y_loc [2, 64, 256, 256]):
SBUF partition dim = (batch, channel) = 2*64 = 128; free dim = a chunk of
HC input rows * 128 cols.  Per iteration (16 of them, HC=8):
  - 4 DMAs (one per subband) load T [128p, band*HC*128]; the DRAM-side
    inner run is HC*512B = 4KB contiguous (channel planes are contiguous)
  - stage 1 (GpSimd tensor_tensor):  U0|V0 = (LL|HL)+(LH|HH),
    U1|V1 = (LL|HL)-(LH|HH)  — frees the Vector engine for stage 2
  - stage 2 (DVE tensor_tensor): out[2i+r, 2j+s] = U_r +- V_r written with
    stride-2 column interleave into OUT [128p, i*512 + r*256 + 2j+s]
  - * 0.5 in place on the contiguous OUT tile (ScalarE ACTIVATE)
  - 1 DMA stores OUT; output rows are consecutive per (batch, channel) so
    the DRAM inner run is 2*HC*256*4B = 16KB contiguous
HBM traffic per core = 33.5 MB in + 33.5 MB out -> ~190 us roofline at
~358 GB/s per-NC HBM bandwidth.

This container's walrus build supports only ONE semaphore wait per
instruction; Tile emits multi-wait instructions (incl. the final drain), so
after TileContext exit we redistribute extra waits onto single-wait NOPs
inserted before the instruction on the same engine.
"""

import numpy as np

import concourse.bass as bass
import concourse.mybir as mybir
from concourse.tile import TileContext
from concourse.bass_utils import run_bass_kernel_spmd

N_CORES = 8
B, C4, H, W = 16, 256, 128, 128
CH = C4 // 4          # 64 output channels
B_LOC = B // N_CORES  # 2 batches per core
HC = 16               # input rows per tile iteration
F32 = mybir.dt.float32


def _split_multi_waits(nc):
    """Move extra semaphore waits onto single-wait NOPs placed immediately
    before the over-subscribed instruction (same engine, so per-engine
    program order is preserved)."""
    n_split = 0
    for f in nc.m.functions:
        for blk in f.blocks:
            il = blk.instructions
            new_list = []
            for inst in il:
                si = getattr(inst, "sync_info", None)
                ow = si.on_wait if si is not None else None
                if ow and len(ow) > 1:
                    extra = list(ow[:-1])
                    del ow[:-1]
                    for w in extra:
                        n_split += 1
                        new_list.append(
                            mybir.InstNoOp(
                                name=f"{inst.name}-waitsplit-{n_split}",
                                engine=inst.engine,
                                sync_info=mybir.SyncInfo(on_wait=[w], on_update=[]),
                            )
                        )
                new_list.append(inst)
            il[:] = new_list
    return n_split


def _build_kernel():
    nc = bass.Bass("TRN2")
    x = nc.dram_tensor("x", [B_LOC, C4, H, W], F32, kind="ExternalInput")
    y = nc.dram_tensor("y", [B_LOC, CH, 2 * H, 2 * W], F32, kind="ExternalOutput")

    FB = HC * W          # free elems per band block
    with TileContext(nc) as tc:
        with (
            tc.tile_pool(name="tin", bufs=2) as pin,
            tc.tile_pool(name="tuv", bufs=1) as puv,
            tc.tile_pool(name="tout", bufs=2) as pout,
        ):
            for it in range(H // HC):
                h0 = it * HC
                # ---- load: T [p=(c,b)][band][i][w]
                # partition p = c*2 + b so the DRAM AP's outermost dim has
                # count 64 (the HWDGE engine spray follows the outer source
                # dim; outer count 2 would use only 2 of 16 SDMA engines)
                T = pin.tile([128, 4 * FB], F32, tag="T")
                for band in range(4):
                    nc.sync.dma_start(
                        out=T[:, band * FB : (band + 1) * FB],
                        in_=x[:, band * CH : (band + 1) * CH, h0 : h0 + HC, :]
                        .rearrange("b c h w -> c b (h w)"),
                    )
                # ---- stage 1: vertical butterfly
                # band = b1*2 + b0: LL=00 LH=01 HL=10 HH=11
                # in0 = (LL, HL) [b0=0], in1 = (LH, HH) [b0=1]
                UV = puv.tile([128, 4 * FB], F32, tag="UV")  # [r][U|V][i][w]
                Tb = T[:].rearrange("p (b1 b0 x) -> p b1 b0 x", b1=2, b0=2)
                in0 = Tb[:, :, 0]
                in1 = Tb[:, :, 1]
                UVr = UV[:].rearrange("p (r x) -> p r x", r=2)
                out0 = UVr[:, 0].rearrange("p (pair x) -> p pair x", pair=2)
                out1 = UVr[:, 1].rearrange("p (pair x) -> p pair x", pair=2)
                nc.vector.tensor_add(out=out0, in0=in0, in1=in1)  # U0|V0
                nc.vector.tensor_sub(out=out1, in0=in0, in1=in1)  # U1|V1
                # ---- stage 2 (DVE): horizontal butterfly + column interleave
                # OUT free layout [i][r][col], col = 2j+s.  Keep every AP at
                # <=2 free dims — 3-free-dim strided DVE ops run ~2x slower.
                OUT = pout.tile([128, 2 * HC * 2 * W], F32, tag="OUT")
                OUTv = OUT[:].rearrange(
                    "p (i r j s) -> p i r j s", i=HC, r=2, j=W, s=2
                )
                UVv = UV[:].rearrange(
                    "p (r pair i w) -> p r pair i w", r=2, pair=2, i=HC
                )
                for r in range(2):
                    u = UVv[:, r, 0]
                    v = UVv[:, r, 1]
                    nc.vector.tensor_add(out=OUTv[:, :, r, :, 0], in0=u, in1=v)
                    nc.vector.tensor_sub(out=OUTv[:, :, r, :, 1], in0=u, in1=v)
                # ---- * 0.5 in place on the contiguous tile (ScalarE/ACT)
                nc.scalar.mul(OUT[:], OUT[:], 0.5)
                # ---- store rows 2*h0 .. 2*h0+2*HC-1 (consecutive)
                nc.sync.dma_start(
                    out=y[:, :, 2 * h0 : 2 * h0 + 2 * HC, :]
                    .rearrange("b c h w -> c b (h w)"),
                    in_=OUT[:],
                )

    _split_multi_waits(nc)
    return nc


_NC_CACHE = None


def _get_nc():
    global _NC_CACHE
    if _NC_CACHE is None:
        _NC_CACHE = _build_kernel()
    return _NC_CACHE


def run_sharded(x, trace=False, **kwargs):
    assert x.shape == (B, C4, H, W) and x.dtype == np.float32
    nc = _get_nc()
    in_maps = [
        {"x": np.ascontiguousarray(x[i * B_LOC : (i + 1) * B_LOC])}
        for i in range(N_CORES)
    ]
    res = run_bass_kernel_spmd(
        nc, in_maps, core_ids=list(range(N_CORES)), trace=trace, **kwargs
    )
    out = np.concatenate([r["y"] for r in res.results], axis=0)
    return out, res


def kernel(x):
    out, _ = run_sharded(np.asarray(x))
    return out

